# revision 1
# baseline (speedup 1.0000x reference)
"""Self-contained Trainium2 Bass kernel for a 4-layer GCN (nn_GCN4).

Strategy (8 NeuronCores, SPMD):
- Nodes are placed (host-side) into 8 slabs of 6272 slots (49 blocks of
  128), balanced so every 128-dst block has a near-equal number of
  incoming edges.
- Per layer: transform locally (dense matmul on the core's 6272 nodes),
  AllGather the bf16 transformed table, then aggregate: dma_gather source
  rows per edge, build a one-hot scatter matrix on the vector engine
  (iota == rel), and scatter-add via TensorE matmul into PSUM.
- Gathers fetch 512-byte elements (a PAIR of adjacent table rows, the
  table viewed as [NPAD/2, 256]): pair indices fit int16 (no lo/hi table
  split) and 512B descriptors use the DMA bus at full efficiency. A
  parity-split one-hot (S_even/S_odd) routes each half of the pair; one
  slot can carry one even- and one odd-parity edge sharing a descriptor.
- Self-loop edges never enter the gather: their contribution (the local
  node's own table row) is injected with an identity matmul from
  SBUF-resident local tables.
- Symmetric normalization deg^-1/2 factors are folded into the table
  (pre-scale) and the activation epilogues (post-scale); the bias is
  injected as a rank-1 "bias wave" matmul so ReLU commutes with the
  deferred scale.
"""
import math
import numpy as np

import concourse.bass as bass
import concourse.bacc as bacc
import concourse.mybir as mybir
import concourse.tile as tile
from concourse import bass_utils

BF = mybir.dt.np(mybir.dt.bfloat16)


class Cfg:
    def __init__(self, N, E, R, NB, group=4):
        self.N = N          # real nodes
        self.E = E          # directed edges (before self loops)
        self.R = R          # cores
        self.NB = NB        # 128-row blocks per core
        self.SLOTS = NB * 128
        self.NPAD = R * self.SLOTS
        self.NBG = R * NB   # global blocks
        self.GROUP = group  # blocks per gather group
        self.F_IN = 128
        self.H1 = 256
        self.H2 = 128
        self.H3 = 64
        self.C = 40
        assert self.N <= self.NPAD
        assert self.NPAD // 2 <= 32768  # pair indices must fit int16


REAL = Cfg(N=50000, E=800000, R=8, NB=49)


# ----------------------------------------------------------------------------
# Host preprocessing
# ----------------------------------------------------------------------------

def preprocess(cfg, x, edge_index, W1, b1, W2, b2, W3, b3, W4, b4):
    N, R, NB = cfg.N, cfg.R, cfg.NB
    SLOTS, NPAD, NBG = cfg.SLOTS, cfg.NPAD, cfg.NBG

    src = np.asarray(edge_index[0], np.int64)
    dst = np.asarray(edge_index[1], np.int64)

    # degree on target index including self-loops (PyG default)
    deg = (np.bincount(dst, minlength=N) + 1).astype(np.float64)
    dinv = 1.0 / np.sqrt(deg)
    rdeg = np.sqrt(deg)

    # --- balanced block assignment: round-based LPT greedy on in-degree -----
    indeg = np.bincount(dst, minlength=N).astype(np.int64)   # excl self loop
    order = np.argsort(-indeg, kind="stable")
    pos = np.full(N, -1, np.int64)
    blk_load = np.zeros(NBG, np.int64)
    blk_cnt = np.zeros(NBG, np.int64)
    blk_of = np.empty(N, np.int64)
    nrounds = (N + NBG - 1) // NBG
    for r in range(nrounds):
        chunk = order[r * NBG:(r + 1) * NBG]
        avail = np.where(blk_cnt < 128)[0]
        la = np.argsort(blk_load[avail], kind="stable")
        tgt = avail[la[:len(chunk)]]
        blk_of[chunk] = tgt
        blk_load[tgt] += indeg[chunk]
        blk_cnt[tgt] += 1
    o = np.argsort(blk_of, kind="stable")
    nodes_s = np.arange(N)[o]
    blk_s = blk_of[o]
    slot = np.arange(N) - np.searchsorted(blk_s, blk_s)
    pos[nodes_s] = blk_s * 128 + slot

    inv_pos = np.full(NPAD, -1, np.int64)
    inv_pos[pos] = np.arange(N)

    # --- edge slots: pair-packed, parity-split, deduped ---------------------
    p_src = pos[src]
    p_dst = pos[dst]
    bg = p_dst >> 7
    rel = (p_dst & 127).astype(np.float32)
    gidx = p_src >> 1
    par = (p_src & 1).astype(np.int64)

    # group edges by (bg, gidx); within a group, even/odd edges are ranked
    # separately: slot k of the group holds even-edge k and odd-edge k.
    gkey = bg * (NPAD // 2) + gidx
    eorder = np.lexsort((par, gkey))
    gkey_s = gkey[eorder]
    par_s = par[eorder]
    M = len(gkey_s)
    kp = gkey_s * 2 + par_s
    newkp = np.empty(M, bool)
    newkp[0] = True
    newkp[1:] = kp[1:] != kp[:-1]
    firstk = np.flatnonzero(newkp)
    r3 = np.arange(M) - np.repeat(firstk, np.diff(np.concatenate([firstk, [M]])))
    newg = np.empty(M, bool)
    newg[0] = True
    newg[1:] = gkey_s[1:] != gkey_s[:-1]
    gid = np.cumsum(newg) - 1              # dense group id per edge
    ngroups = int(gid[-1]) + 1
    cnt_ep = np.zeros(2 * ngroups, np.int64)
    np.add.at(cnt_ep, gid * 2 + par_s, 1)
    nslot_g = np.maximum(cnt_ep[0::2], cnt_ep[1::2])
    bg_g = bg[eorder][newg]
    # base slot of each group within its block (groups sorted by bg)
    cums = np.cumsum(nslot_g) - nslot_g
    blk_first = np.zeros(NBG, np.int64)
    newb = np.empty(ngroups, bool)
    newb[0] = True
    newb[1:] = bg_g[1:] != bg_g[:-1]
    blk_first[bg_g[newb]] = cums[newb]
    base_g = cums - blk_first[bg_g]
    j = base_g[gid] + r3                   # slot within block
    cnt_b = np.zeros(NBG, np.int64)
    np.add.at(cnt_b, bg_g, nslot_g)
    T = int(math.ceil(cnt_b.max() / 128))

    t_s = j >> 7
    pp = j & 127
    A_idx = np.zeros((NBG, T, 128), np.int16)
    A_rel = np.full((NBG, 2 * T, 128), -1.0, np.float32)  # interleaved E/O
    bs_ = bg[eorder]
    A_idx[bs_, t_s, pp] = gidx[eorder].astype(np.int16)
    A_rel[bs_, 2 * t_s + par_s, pp] = rel[eorder]

    # --- per-position node attributes --------------------------------------
    dinv_pos = np.zeros(NPAD, np.float64)
    rdeg_pos = np.zeros(NPAD, np.float64)
    dinv_pos[pos] = dinv
    rdeg_pos[pos] = rdeg

    xp = np.zeros((NPAD, cfg.F_IN), np.float32)
    xp[pos] = np.asarray(x, np.float32) * dinv[:, None]
    xp = xp.astype(BF)

    def wrap(a):
        # a: [nblk, T, 128] int -> wrapped [128, nblk*T*8] int16
        flat = a.reshape(-1)
        w = flat.reshape(-1, 16).T            # [16, n/16]
        return np.tile(w, (8, 1)).astype(np.int16)

    ident = np.eye(128, dtype=np.float32)

    in_maps = []
    for r in range(R):
        bl = slice(r * NB, (r + 1) * NB)
        sl = slice(r * SLOTS, (r + 1) * SLOTS)
        grel = A_rel[bl].transpose(2, 0, 1).reshape(128, NB * 2 * T)
        m = {
            "xp": xp,
            "idx": wrap(A_idx[bl]),
            "grel": grel.astype(BF),
            "iota": np.tile(np.arange(128, dtype=np.float32), (128, 1)).astype(BF),
            "ident": ident.astype(BF),
            "dinvp": dinv_pos[sl].reshape(NB, 128).T.astype(np.float32).copy(),
            "dinv2p": (dinv_pos[sl] ** 2).reshape(NB, 128).T.astype(np.float32).copy(),
            "rdegb": rdeg_pos[sl].reshape(1, SLOTS).astype(BF),
            "w1": np.asarray(W1, np.float32).astype(BF),
            "w2": np.asarray(W2, np.float32).reshape(2, 128, cfg.H2)
                    .transpose(1, 0, 2).astype(BF),
            "w3": np.asarray(W3, np.float32).astype(BF),
            "w4": np.asarray(W4, np.float32).astype(BF),
            "b1": np.asarray(b1, np.float32).reshape(1, -1).astype(BF),
            "b2": np.asarray(b2, np.float32).reshape(1, -1).astype(BF),
            "b3": np.asarray(b3, np.float32).reshape(1, -1).astype(BF),
            "b4": np.asarray(b4, np.float32).reshape(1, -1).astype(BF),
        }
        in_maps.append(m)

    struct = (T,)
    return in_maps, struct, inv_pos


# ----------------------------------------------------------------------------
# Bass program
# ----------------------------------------------------------------------------

def build(cfg, T, reps=1, skip=(), nq=4, spk=False):
    NB, SLOTS, NPAD = cfg.NB, cfg.SLOTS, cfg.NPAD
    NPAIR = NPAD // 2
    bf16 = mybir.dt.bfloat16
    f32 = mybir.dt.float32
    RELU = mybir.ActivationFunctionType.Relu
    COPY = mybir.ActivationFunctionType.Copy

    groups = []
    b0 = 0
    while b0 < NB:
        nbk = min(cfg.GROUP, NB - b0)
        groups.append((b0, nbk))
        b0 += nbk

    nc = bacc.Bacc("TRN2", target_bir_lowering=False, debug=False,
                   num_devices=cfg.R, num_swdge_queues=nq)
    rg = [list(range(cfg.R))]
    qc = [0]

    def nxq():
        q = qc[0] % nq
        qc[0] += 1
        return q

    # I/O
    xp_d = nc.dram_tensor("xp", [NPAD, cfg.F_IN], bf16, kind="ExternalInput")
    idx_d = nc.dram_tensor("idx", [128, NB * T * 8], mybir.dt.int16, kind="ExternalInput")
    grel_d = nc.dram_tensor("grel", [128, NB * 2 * T], bf16, kind="ExternalInput")
    iota_d = nc.dram_tensor("iota", [128, 128], bf16, kind="ExternalInput")
    ident_d = nc.dram_tensor("ident", [128, 128], bf16, kind="ExternalInput")
    dinvp_d = nc.dram_tensor("dinvp", [128, NB], f32, kind="ExternalInput")
    dinv2p_d = nc.dram_tensor("dinv2p", [128, NB], f32, kind="ExternalInput")
    rdegb_d = nc.dram_tensor("rdegb", [1, SLOTS], bf16, kind="ExternalInput")
    w1_d = nc.dram_tensor("w1", [128, cfg.H1], bf16, kind="ExternalInput")
    w2_d = nc.dram_tensor("w2", [128, 2, cfg.H2], bf16, kind="ExternalInput")
    w3_d = nc.dram_tensor("w3", [cfg.H2, cfg.H3], bf16, kind="ExternalInput")
    w4_d = nc.dram_tensor("w4", [cfg.H3, cfg.C], bf16, kind="ExternalInput")
    b1_d = nc.dram_tensor("b1", [1, cfg.H1], bf16, kind="ExternalInput")
    b2_d = nc.dram_tensor("b2", [1, cfg.H2], bf16, kind="ExternalInput")
    b3_d = nc.dram_tensor("b3", [1, cfg.H3], bf16, kind="ExternalInput")
    b4_d = nc.dram_tensor("b4", [1, cfg.C], bf16, kind="ExternalInput")
    out_d = nc.dram_tensor("out", [SLOTS, cfg.C], f32, kind="ExternalOutput")

    shared = "Shared" if cfg.R > 4 else "Local"
    ag2in = nc.dram_tensor("ag2in", [SLOTS, 128], bf16, kind="Internal")
    ag2out = nc.dram_tensor("ag2out", [NPAD, 128], bf16, kind="Internal", addr_space=shared)
    ag3in = nc.dram_tensor("ag3in", [SLOTS, 128], bf16, kind="Internal")
    ag3out = nc.dram_tensor("ag3out", [NPAD, 128], bf16, kind="Internal", addr_space=shared)
    ag4in = nc.dram_tensor("ag4in", [SLOTS, 128], bf16, kind="Internal")
    ag4out = nc.dram_tensor("ag4out", [NPAD, 128], bf16, kind="Internal", addr_space=shared)

    with tile.TileContext(nc) as tc:
        with (
            tc.tile_pool(name="res", bufs=1) as res,          # resident
            tc.tile_pool(name="idxp", bufs=3) as idxp,
            tc.tile_pool(name="gat", bufs=2) as gat,
            tc.tile_pool(name="xbp", bufs=2) as xbp,
            tc.tile_pool(name="sp", bufs=3) as sp,
            tc.tile_pool(name="epi", bufs=3) as epi,
            tc.tile_pool(name="aps", bufs=3, space="PSUM") as aps,
            tc.tile_pool(name="tps", bufs=2, space="PSUM") as tps,
        ):
            def _kbody():
                # residents
                grel_t = res.tile([128, NB * 2 * T], bf16)
                iota_t = res.tile([128, 128], bf16)
                ident_t = res.tile([128, 128], bf16)
                dinvp_t = res.tile([128, NB], f32)
                dinv2p_t = res.tile([128, NB], f32)
                rdegb_t = res.tile([1, SLOTS], bf16)
                w1_t = res.tile([128, cfg.H1], bf16)
                w2_t = res.tile([128, 2, cfg.H2], bf16)
                w3_t = res.tile([cfg.H2, cfg.H3], bf16)
                w4_t = res.tile([cfg.H3, cfg.C], bf16)
                b1_t = res.tile([1, cfg.H1], bf16)
                b2_t = res.tile([1, cfg.H2], bf16)
                b3_t = res.tile([1, cfg.H3], bf16)
                b4_t = res.tile([1, cfg.C], bf16)
                for t, d in ((grel_t, grel_d), (iota_t, iota_d), (ident_t, ident_d),
                             (dinvp_t, dinvp_d), (dinv2p_t, dinv2p_d), (rdegb_t, rdegb_d),
                             (w1_t, w1_d), (w2_t, w2_d), (w3_t, w3_d), (w4_t, w4_d),
                             (b1_t, b1_d), (b2_t, b2_d), (b3_t, b3_d), (b4_t, b4_d)):
                    nc.sync.dma_start(t[:], d[:])

                agg1T = res.tile([128, SLOTS], bf16)   # L1 raw aggregate, feature-major
                h1T0 = res.tile([128, SLOTS], bf16)    # relu(agg1T@W1 + bias), j-tile 0
                h1T1 = res.tile([128, SLOTS], bf16)
                h2T = res.tile([128, SLOTS], bf16)
                h3T = res.tile([cfg.H3, SLOTS], bf16)
                loc2 = res.tile([128, NB, 128], bf16)  # local tables, node-major
                loc3 = res.tile([128, NB, cfg.H3], bf16)
                loc4 = res.tile([128, NB, cfg.C], bf16)

                iota_b = iota_t[:].unsqueeze(1).broadcast_to([128, 2 * T, 128])

                def aggregate(table_d, FW, out_cb, bias_wave, self_lhsT):
                    """Generic aggregation layer (feature-major psum [FW, 128]).
                    table_d: DRAM table [NPAD, 128] bf16, gathered as pairs
                    FW: lhsT feature width used
                    out_cb(b, psum): epilogue for block b
                    bias_wave(b, psum): may start accumulation
                    self_lhsT(g0, nbk) -> (k -> AP [128, FW]) node-major local
                      rows for the self-loop identity injection."""
                    pair_ap = table_d.reshape([NPAIR, 256])[:, :]
                    for (g0, nbk) in groups:
                        g = gat.tile([128, cfg.GROUP * T, 256], bf16, tag="g")
                        ix = idxp.tile([128, cfg.GROUP * T * 8], mybir.dt.int16, tag="ix")
                        ni = nbk * T * 128
                        nc.sync.dma_start(ix[:, :nbk * T * 8],
                                          idx_d[:, g0 * T * 8:(g0 + nbk) * T * 8])
                        if "gather" not in skip:
                            nc.gpsimd.dma_gather(
                                g[:, :nbk * T, :], pair_ap, ix[:, :nbk * T * 8],
                                num_idxs=ni, num_idxs_reg=ni, elem_size=256,
                                single_packet=spk, queue_num=nxq())
                        else:
                            tbl3 = table_d.reshape([128, NPAD // 128 // 2, 256])
                            nc.sync.dma_start(g[:, :nbk * T, :], tbl3[:, 0:nbk * T, :])
                        selfb = self_lhsT(g0, nbk)
                        for k in range(nbk):
                            b = g0 + k
                            S = sp.tile([128, 2 * T, 128], bf16, tag="S")
                            if "sbuild" not in skip:
                                nc.vector.tensor_tensor(
                                    S[:],
                                    iota_b,
                                    grel_t[:, b * 2 * T:(b + 1) * 2 * T]
                                        .unsqueeze(2).broadcast_to([128, 2 * T, 128]),
                                    mybir.AluOpType.is_equal)
                            psum = aps.tile([FW, 128], f32, tag="agg")
                            started = bias_wave(b, psum)
                            nc.tensor.matmul(psum[:], selfb(k), ident_t[:],
                                             start=not started, stop=False)
                            for t in range(T):
                                if "aggmm" in skip and t > 0:
                                    continue
                                last = (t == T - 1) or ("aggmm" in skip)
                                nc.tensor.matmul(psum[:], g[:, k * T + t, 0:FW],
                                                 S[:, 2 * t, :],
                                                 start=False, stop=False)
                                nc.tensor.matmul(psum[:], g[:, k * T + t, 128:128 + FW],
                                                 S[:, 2 * t + 1, :],
                                                 start=False, stop=last)
                            out_cb(b, psum)

                # ---------------- L1 aggregation (table = xp) ----------------
                def l1_out(b, psum):
                    nc.vector.tensor_copy(agg1T[:, b * 128:(b + 1) * 128], psum[:])

                def l1_self(g0, nbk):
                    xb = xbp.tile([128, cfg.GROUP, 128], bf16, tag="xb")
                    for k in range(nbk):
                        nc.sync.dma_start(xb[:, k, :],
                                          xp_d[(g0 + k) * 128:(g0 + k + 1) * 128, :])
                    return lambda k: xb[:, k, :]

                aggregate(xp_d, 128, l1_out, lambda b, p: False, l1_self)

                # ---------------- L1 transform -> h1T ----------------
                v0 = 0
                while v0 < SLOTS:
                    vsz = min(512, SLOTS - v0)
                    for j in range(2):
                        pt = tps.tile([128, 512], f32, tag="tps")
                        nc.tensor.matmul(pt[:, :vsz], w1_t[:, j * 128:(j + 1) * 128],
                                         agg1T[:, v0:v0 + vsz], start=True, stop=False)
                        nc.tensor.matmul(pt[:, :vsz], b1_t[0:1, j * 128:(j + 1) * 128],
                                         rdegb_t[0:1, v0:v0 + vsz], start=False, stop=True)
                        h = h1T0 if j == 0 else h1T1
                        nc.scalar.activation(h[:, v0:v0 + vsz], pt[:, :vsz], RELU)
                    v0 += vsz

                # ---------------- L2 transform -> loc2/ag2in; AllGather ------
                for b in range(NB):
                    bs = slice(b * 128, (b + 1) * 128)
                    pt = tps.tile([128, 512], f32, tag="tps")
                    nc.tensor.matmul(pt[:, :128], h1T0[:, bs], w2_t[:, 0, :], start=True, stop=False)
                    nc.tensor.matmul(pt[:, :128], h1T1[:, bs], w2_t[:, 1, :], start=False, stop=True)
                    nc.scalar.activation(loc2[:, b, :], pt[:, :128], COPY,
                                         scale=dinv2p_t[:, b:b + 1])
                    nc.sync.dma_start(ag2in[bs, :], loc2[:, b, :])
                if "coll" in skip:
                    nc.sync.dma_start(ag2out[0:SLOTS, :], ag2in[:])
                else:
                    nc.gpsimd.collective_compute(
                        "AllGather", mybir.AluOpType.bypass,
                        replica_groups=rg, ins=[ag2in[:]], outs=[ag2out[:]])

                # ---------------- L2 aggregation -> h2T ----------------
                def l2_bias(b, psum):
                    nc.tensor.matmul(psum[:], b2_t[:], rdegb_t[0:1, b * 128:(b + 1) * 128],
                                     start=True, stop=False)
                    return True

                def l2_out(b, psum):
                    nc.scalar.activation(h2T[:, b * 128:(b + 1) * 128], psum[:], RELU)

                aggregate(ag2out, 128, l2_out, l2_bias,
                          lambda g0, nbk: (lambda k: loc2[:, g0 + k, :]))

                # ---------------- L3 transform -> loc3/ag3in; AllGather ------
                for b in range(NB):
                    bs = slice(b * 128, (b + 1) * 128)
                    pt = tps.tile([128, 512], f32, tag="tps")
                    nc.tensor.matmul(pt[:, :cfg.H3], h2T[:, bs], w3_t[:], start=True, stop=True)
                    nc.scalar.activation(loc3[:, b, :], pt[:, :cfg.H3], COPY,
                                         scale=dinv2p_t[:, b:b + 1])
                    # pad cols of the table are never consumed (FW-sliced lhsT)
                    nc.sync.dma_start(ag3in[bs, 0:cfg.H3], loc3[:, b, :])
                if "coll" in skip:
                    nc.sync.dma_start(ag3out[0:SLOTS, :], ag3in[:])
                else:
                    nc.gpsimd.collective_compute(
                        "AllGather", mybir.AluOpType.bypass,
                        replica_groups=rg, ins=[ag3in[:]], outs=[ag3out[:]])

                # ---------------- L3 aggregation -> h3T ----------------
                def l3_bias(b, psum):
                    nc.tensor.matmul(psum[:], b3_t[:], rdegb_t[0:1, b * 128:(b + 1) * 128],
                                     start=True, stop=False)
                    return True

                def l3_out(b, psum):
                    nc.scalar.activation(h3T[:, b * 128:(b + 1) * 128], psum[:], RELU)

                aggregate(ag3out, cfg.H3, l3_out, l3_bias,
                          lambda g0, nbk: (lambda k: loc3[:, g0 + k, :]))

                # ---------------- L4 transform -> loc4/ag4in; AllGather ------
                for b in range(NB):
                    bs = slice(b * 128, (b + 1) * 128)
                    pt = tps.tile([128, 512], f32, tag="tps")
                    nc.tensor.matmul(pt[:, :cfg.C], h3T[:, bs], w4_t[:], start=True, stop=True)
                    nc.scalar.activation(loc4[:, b, :], pt[:, :cfg.C], COPY,
                                         scale=dinv2p_t[:, b:b + 1])
                    nc.sync.dma_start(ag4in[bs, 0:cfg.C], loc4[:, b, :])
                if "coll" in skip:
                    nc.sync.dma_start(ag4out[0:SLOTS, :], ag4in[:])
                else:
                    nc.gpsimd.collective_compute(
                        "AllGather", mybir.AluOpType.bypass,
                        replica_groups=rg, ins=[ag4in[:]], outs=[ag4out[:]])

                # ---------------- L4 aggregation (node-major) -> out ----------
                pair_ap4 = ag4out.reshape([NPAIR, 256])[:, :]
                for (g0, nbk) in groups:
                    g = gat.tile([128, cfg.GROUP * T, 256], bf16, tag="g")
                    ix = idxp.tile([128, cfg.GROUP * T * 8], mybir.dt.int16, tag="ix")
                    ni = nbk * T * 128
                    nc.sync.dma_start(ix[:, :nbk * T * 8],
                                      idx_d[:, g0 * T * 8:(g0 + nbk) * T * 8])
                    if "gather" not in skip:
                        nc.gpsimd.dma_gather(
                            g[:, :nbk * T, :], pair_ap4, ix[:, :nbk * T * 8],
                            num_idxs=ni, num_idxs_reg=ni, elem_size=256,
                            single_packet=spk, queue_num=nxq())
                    else:
                        tbl3 = ag4out.reshape([128, NPAD // 128 // 2, 256])
                        nc.sync.dma_start(g[:, :nbk * T, :], tbl3[:, 0:nbk * T, :])
                    for k in range(nbk):
                        b = g0 + k
                        S = sp.tile([128, 2 * T, 128], bf16, tag="S")
                        if "sbuild" not in skip:
                            nc.vector.tensor_tensor(
                                S[:], iota_b,
                                grel_t[:, b * 2 * T:(b + 1) * 2 * T]
                                    .unsqueeze(2).broadcast_to([128, 2 * T, 128]),
                                mybir.AluOpType.is_equal)
                        psum = aps.tile([128, cfg.C], f32, tag="agg4")
                        nc.tensor.matmul(psum[:], rdegb_t[0:1, b * 128:(b + 1) * 128],
                                         b4_t[:], start=True, stop=False)
                        nc.tensor.matmul(psum[:], ident_t[:], loc4[:, b, :],
                                         start=False, stop=False)
                        for t in range(T):
                            if "aggmm" in skip and t > 0:
                                continue
                            last = (t == T - 1) or ("aggmm" in skip)
                            nc.tensor.matmul(psum[:], S[:, 2 * t, :],
                                             g[:, k * T + t, 0:cfg.C],
                                             start=False, stop=False)
                            nc.tensor.matmul(psum[:], S[:, 2 * t + 1, :],
                                             g[:, k * T + t, 128:128 + cfg.C],
                                             start=False, stop=last)
                        o = epi.tile([128, cfg.C], f32, tag="o4")
                        nc.scalar.activation(o[:], psum[:], COPY, scale=dinvp_t[:, b:b + 1])
                        nc.sync.dma_start(out_d[b * 128:(b + 1) * 128, :], o[:])

            for _rep in range(reps):
                _kbody()

    nc.compile()
    return nc


# ----------------------------------------------------------------------------
# Driver
# ----------------------------------------------------------------------------

_CACHE = {}


def run(cfg, inputs, trace=False):
    in_maps, struct, inv_pos = preprocess(cfg, **inputs)
    key = (cfg.N, cfg.E, cfg.R, cfg.NB) + struct
    if key not in _CACHE:
        _CACHE[key] = build(cfg, *struct)
    nc = _CACHE[key]
    res = bass_utils.run_bass_kernel_spmd(
        nc, in_maps, core_ids=list(range(cfg.R)), trace=trace)
    outs = [res.results[r]["out"] for r in range(cfg.R)]
    full = np.concatenate(outs, axis=0)          # [NPAD, C]
    out = np.empty((cfg.N, cfg.C), np.float32)
    valid = inv_pos >= 0
    out[inv_pos[valid]] = full[valid]
    return out, res


def kernel(**inputs):
    out, _ = run(REAL, inputs)
    return out



# revision 6
# speedup vs baseline: 2.3453x; 2.3453x over previous
"""Trainium2 Bass kernel for nn_GCN4 — v2 (fp8 tables + DoubleRow scatter).

Differences from v1 (kernel.py):
- Gathered tables (xp, ag2/3/4) stored fp8e4m3: pair elements are 256B
  (half the gather traffic), AllGather bytes halve, SBUF halves.
- Scatter-add matmuls use MatmulPerfMode.DoubleRow: one matmul per
  slot-block covers BOTH pair parities (K=256) at 0.5 cycles/row — the
  [even|odd] pair layout and adjacent S planes are exactly the DoubleRow
  operand shapes.
- L1 self-loop rows come from a per-core `xloc` input (the core's own
  slab) — v1 read global rows [0:SLOTS] (core 0's slab) on every core.
- One-hot S built in fp8 (is_equal writes 0.0/1.0 exactly).
- L3/L4 local tables are zero-padded to 128 cols so the fp8 pair
  elements never contain uninitialized bytes.
"""
import math
import numpy as np

import concourse.bass as bass
import concourse.bacc as bacc
import concourse.mybir as mybir
import concourse.tile as tile
from concourse import bass_utils

BF = mybir.dt.np(mybir.dt.bfloat16)
F8 = mybir.dt.np(mybir.dt.float8e4)


class Cfg:
    def __init__(self, N, E, R, NB, group=4):
        self.N = N
        self.E = E
        self.R = R
        self.NB = NB
        self.SLOTS = NB * 128
        self.NPAD = R * self.SLOTS
        self.NBG = R * NB
        self.GROUP = group
        self.F_IN = 128
        self.H1 = 256
        self.H2 = 128
        self.H3 = 64
        self.C = 40
        assert self.N <= self.NPAD
        assert self.NPAD // 2 <= 32768


REAL = Cfg(N=50000, E=800000, R=8, NB=49, group=2)


# ----------------------------------------------------------------------------
# Host preprocessing
# ----------------------------------------------------------------------------

def preprocess(cfg, x, edge_index, W1, b1, W2, b2, W3, b3, W4, b4):
    N, R, NB = cfg.N, cfg.R, cfg.NB
    SLOTS, NPAD, NBG = cfg.SLOTS, cfg.NPAD, cfg.NBG

    src = np.asarray(edge_index[0], np.int64)
    dst = np.asarray(edge_index[1], np.int64)

    deg = (np.bincount(dst, minlength=N) + 1).astype(np.float64)
    dinv = 1.0 / np.sqrt(deg)
    rdeg = np.sqrt(deg)

    # balanced block assignment: round-based LPT greedy on in-degree
    indeg = np.bincount(dst, minlength=N).astype(np.int64)
    order = np.argsort(-indeg, kind="stable")
    pos = np.full(N, -1, np.int64)
    blk_load = np.zeros(NBG, np.int64)
    blk_cnt = np.zeros(NBG, np.int64)
    blk_of = np.empty(N, np.int64)
    nrounds = (N + NBG - 1) // NBG
    for r in range(nrounds):
        chunk = order[r * NBG:(r + 1) * NBG]
        avail = np.where(blk_cnt < 128)[0]
        la = np.argsort(blk_load[avail], kind="stable")
        tgt = avail[la[:len(chunk)]]
        blk_of[chunk] = tgt
        blk_load[tgt] += indeg[chunk]
        blk_cnt[tgt] += 1
    o = np.argsort(blk_of, kind="stable")
    nodes_s = np.arange(N)[o]
    blk_s = blk_of[o]
    slot = np.arange(N) - np.searchsorted(blk_s, blk_s)
    pos[nodes_s] = blk_s * 128 + slot

    inv_pos = np.full(NPAD, -1, np.int64)
    inv_pos[pos] = np.arange(N)

    # edge slots: pair-packed, parity-split, deduped
    p_src = pos[src]
    p_dst = pos[dst]
    bg = p_dst >> 7
    rel = (p_dst & 127).astype(np.float32)
    gidx = p_src >> 1
    par = (p_src & 1).astype(np.int64)

    gkey = bg * (NPAD // 2) + gidx
    eorder = np.lexsort((par, gkey))
    gkey_s = gkey[eorder]
    par_s = par[eorder]
    M = len(gkey_s)
    kp = gkey_s * 2 + par_s
    newkp = np.empty(M, bool)
    newkp[0] = True
    newkp[1:] = kp[1:] != kp[:-1]
    firstk = np.flatnonzero(newkp)
    r3 = np.arange(M) - np.repeat(firstk, np.diff(np.concatenate([firstk, [M]])))
    newg = np.empty(M, bool)
    newg[0] = True
    newg[1:] = gkey_s[1:] != gkey_s[:-1]
    gid = np.cumsum(newg) - 1
    ngroups = int(gid[-1]) + 1
    cnt_ep = np.zeros(2 * ngroups, np.int64)
    np.add.at(cnt_ep, gid * 2 + par_s, 1)
    nslot_g = np.maximum(cnt_ep[0::2], cnt_ep[1::2])
    bg_g = bg[eorder][newg]
    cums = np.cumsum(nslot_g) - nslot_g
    blk_first = np.zeros(NBG, np.int64)
    newb = np.empty(ngroups, bool)
    newb[0] = True
    newb[1:] = bg_g[1:] != bg_g[:-1]
    blk_first[bg_g[newb]] = cums[newb]
    base_g = cums - blk_first[bg_g]
    j = base_g[gid] + r3
    cnt_b = np.zeros(NBG, np.int64)
    np.add.at(cnt_b, bg_g, nslot_g)
    T = int(math.ceil(cnt_b.max() / 128))

    t_s = j >> 7
    pp = j & 127
    A_idx = np.full((NBG, T * 128), -1, np.int32)
    A_rel = np.full((NBG, 2 * T, 128), -1.0, np.float32)
    bs_ = bg[eorder]
    A_idx[bs_, j] = gidx[eorder].astype(np.int32)
    A_rel[bs_, 2 * t_s + par_s, pp] = rel[eorder]
    # fill padding slots with the preceding valid index (keeps gather
    # addresses distinct and locally ascending; S row is zero anyway)
    for bb in range(NBG):
        row = A_idx[bb]
        m_ = row >= 0
        if not m_.all():
            idxs_valid = np.where(m_, np.arange(T * 128), 0)
            np.maximum.accumulate(idxs_valid, out=idxs_valid)
            row[:] = row[idxs_valid]
            if row[0] < 0:
                row[~(row >= 0)] = 0
    A_idx = A_idx.reshape(NBG, T, 128).astype(np.int16)

    dinv_pos = np.zeros(NPAD, np.float64)
    rdeg_pos = np.zeros(NPAD, np.float64)
    dinv_pos[pos] = dinv
    rdeg_pos[pos] = rdeg

    xp = np.zeros((NPAD, cfg.F_IN), np.float32)
    xp[pos] = np.asarray(x, np.float32) * dinv[:, None]
    xp8 = xp.astype(F8)

    def wrap(a):
        flat = a.reshape(-1)
        w = flat.reshape(-1, 16).T
        return np.tile(w, (8, 1)).astype(np.int16)

    ident = np.eye(128, dtype=np.float32)

    in_maps = []
    for r in range(R):
        bl = slice(r * NB, (r + 1) * NB)
        sl = slice(r * SLOTS, (r + 1) * SLOTS)
        grel = A_rel[bl].transpose(2, 0, 1).reshape(128, NB * 2 * T)
        m = {
            "xp": xp8,
            "xloc": xp8[sl].copy(),
            "idx": wrap(A_idx[bl]),
            "grel": grel.astype(BF),
            "iota": np.tile(np.arange(128, dtype=np.float32), (128, 1)).astype(BF),
            "ident": ident.astype(F8),
            "dinvp": dinv_pos[sl].reshape(NB, 128).T.astype(np.float32).copy(),
            "dinv2p": (dinv_pos[sl] ** 2).reshape(NB, 128).T.astype(np.float32).copy(),
            "rdegb": rdeg_pos[sl].reshape(1, SLOTS).astype(BF),
            "w1": np.asarray(W1, np.float32).astype(BF),
            "w2": np.asarray(W2, np.float32).reshape(2, 128, cfg.H2)
                    .transpose(1, 0, 2).astype(BF),
            "w3": np.asarray(W3, np.float32).astype(BF),
            "w4": np.asarray(W4, np.float32).astype(BF),
            "b1": np.asarray(b1, np.float32).reshape(1, -1).astype(BF),
            "b2": np.asarray(b2, np.float32).reshape(1, -1).astype(BF),
            "b3": np.asarray(b3, np.float32).reshape(1, -1).astype(BF),
            "b4": np.asarray(b4, np.float32).reshape(1, -1).astype(BF),
        }
        in_maps.append(m)

    struct = (T,)
    return in_maps, struct, inv_pos


# ----------------------------------------------------------------------------
# Bass program
# ----------------------------------------------------------------------------

def build(cfg, T, reps=1, skip=(), nq=4, spk=False, gat_bufs=6, idx_bufs=6,
          group=None, sd=None, ncache=0):
    NB, SLOTS, NPAD = cfg.NB, cfg.SLOTS, cfg.NPAD
    NPAIR = NPAD // 2
    bf16 = mybir.dt.bfloat16
    fp8 = mybir.dt.float8e4
    f32 = mybir.dt.float32
    RELU = mybir.ActivationFunctionType.Relu
    COPY = mybir.ActivationFunctionType.Copy
    DR = mybir.MatmulPerfMode.DoubleRow

    GROUP = group if group is not None else cfg.GROUP
    groups = []
    b0 = 0
    while b0 < NB:
        nbk = min(GROUP, NB - b0)
        groups.append((b0, nbk))
        b0 += nbk

    nc = bacc.Bacc("TRN2", target_bir_lowering=False, debug=False,
                   num_devices=cfg.R, num_swdge_queues=nq)
    rg = [list(range(cfg.R))]
    qc = [0]

    def nxq():
        q = qc[0] % nq
        qc[0] += 1
        return q

    xp_d = nc.dram_tensor("xp", [NPAD, cfg.F_IN], fp8, kind="ExternalInput")
    xloc_d = nc.dram_tensor("xloc", [SLOTS, cfg.F_IN], fp8, kind="ExternalInput")
    idx_d = nc.dram_tensor("idx", [128, NB * T * 8], mybir.dt.int16, kind="ExternalInput")
    grel_d = nc.dram_tensor("grel", [128, NB * 2 * T], bf16, kind="ExternalInput")
    iota_d = nc.dram_tensor("iota", [128, 128], bf16, kind="ExternalInput")
    ident_d = nc.dram_tensor("ident", [128, 128], fp8, kind="ExternalInput")
    dinvp_d = nc.dram_tensor("dinvp", [128, NB], f32, kind="ExternalInput")
    dinv2p_d = nc.dram_tensor("dinv2p", [128, NB], f32, kind="ExternalInput")
    rdegb_d = nc.dram_tensor("rdegb", [1, SLOTS], bf16, kind="ExternalInput")
    w1_d = nc.dram_tensor("w1", [128, cfg.H1], bf16, kind="ExternalInput")
    w2_d = nc.dram_tensor("w2", [128, 2, cfg.H2], bf16, kind="ExternalInput")
    w3_d = nc.dram_tensor("w3", [cfg.H2, cfg.H3], bf16, kind="ExternalInput")
    w4_d = nc.dram_tensor("w4", [cfg.H3, cfg.C], bf16, kind="ExternalInput")
    b1_d = nc.dram_tensor("b1", [1, cfg.H1], bf16, kind="ExternalInput")
    b2_d = nc.dram_tensor("b2", [1, cfg.H2], bf16, kind="ExternalInput")
    b3_d = nc.dram_tensor("b3", [1, cfg.H3], bf16, kind="ExternalInput")
    b4_d = nc.dram_tensor("b4", [1, cfg.C], bf16, kind="ExternalInput")
    out_d = nc.dram_tensor("out", [SLOTS, cfg.C], f32, kind="ExternalOutput")

    shared = "Shared" if cfg.R > 4 else "Local"
    ag2in = nc.dram_tensor("ag2in", [SLOTS, 128], fp8, kind="Internal")
    ag2out = nc.dram_tensor("ag2out", [NPAD, 128], fp8, kind="Internal", addr_space=shared)
    ag3in = nc.dram_tensor("ag3in", [SLOTS, 128], fp8, kind="Internal")
    ag3out = nc.dram_tensor("ag3out", [NPAD, 128], fp8, kind="Internal", addr_space=shared)
    ag4in = nc.dram_tensor("ag4in", [SLOTS, 128], fp8, kind="Internal")
    ag4out = nc.dram_tensor("ag4out", [NPAD, 128], fp8, kind="Internal", addr_space=shared)

    with tile.TileContext(nc) as tc:
        with (
            tc.tile_pool(name="res", bufs=1) as res,
            tc.tile_pool(name="gat", bufs=gat_bufs) as gat,
            tc.tile_pool(name="xbp", bufs=2) as xbp,
            tc.tile_pool(name="sp", bufs=3) as sp,
            tc.tile_pool(name="epi", bufs=3) as epi,
            tc.tile_pool(name="aps", bufs=3, space="PSUM") as aps,
            tc.tile_pool(name="tps", bufs=2, space="PSUM") as tps,
        ):
            def _kbody():
                grel_t = res.tile([128, NB * 2 * T], bf16)
                iota_t = res.tile([128, 128], bf16)
                ident_t = res.tile([128, 128], fp8)
                dinvp_t = res.tile([128, NB], f32)
                dinv2p_t = res.tile([128, NB], f32)
                rdegb_t = res.tile([1, SLOTS], bf16)
                w1_t = res.tile([128, cfg.H1], bf16)
                w2_t = res.tile([128, 2, cfg.H2], bf16)
                w3_t = res.tile([cfg.H2, cfg.H3], bf16)
                w4_t = res.tile([cfg.H3, cfg.C], bf16)
                b1_t = res.tile([1, cfg.H1], bf16)
                b2_t = res.tile([1, cfg.H2], bf16)
                b3_t = res.tile([1, cfg.H3], bf16)
                b4_t = res.tile([1, cfg.C], bf16)
                idx_t = res.tile([128, NB * T * 8], mybir.dt.int16)
                for t, d in ((idx_t, idx_d), (grel_t, grel_d), (iota_t, iota_d), (ident_t, ident_d),
                             (dinvp_t, dinvp_d), (dinv2p_t, dinv2p_d), (rdegb_t, rdegb_d),
                             (w1_t, w1_d), (w2_t, w2_d), (w3_t, w3_d), (w4_t, w4_d),
                             (b1_t, b1_d), (b2_t, b2_d), (b3_t, b3_d), (b4_t, b4_d)):
                    nc.sync.dma_start(t[:], d[:])

                if ncache:
                    scache = res.tile([128, ncache, 2 * T, 128], fp8)
                else:
                    scache = None
                agg1T = res.tile([128, SLOTS], bf16)
                h1T0 = res.tile([128, SLOTS], bf16)
                h1T1 = res.tile([128, SLOTS], bf16)
                h2T = res.tile([128, SLOTS], bf16)
                h3T = res.tile([cfg.H3, SLOTS], bf16)
                loc2 = res.tile([128, NB, 128], fp8)   # local tables, node-major
                loc3 = res.tile([128, NB, 128], fp8)   # cols 64:128 zeroed
                loc4 = res.tile([128, NB, 128], fp8)   # cols 40:128 zeroed
                nc.vector.memset(loc3[:], 0)
                nc.vector.memset(loc4[:], 0)

                iota_b = iota_t[:].unsqueeze(1).broadcast_to([128, 2 * T, 128])

                def get_S(b, first):
                    if scache is not None and b < ncache:
                        Sap = scache[:, b, :, :]
                        build_it = first
                    else:
                        S_t = sp.tile([128, 2 * T, 128], fp8, tag="S")
                        Sap = S_t[:]
                        build_it = True
                    if build_it:
                        if "sbuild" not in skip:
                            nc.vector.tensor_tensor(
                                Sap,
                                iota_t[:].unsqueeze(1).broadcast_to([128, 2 * T, 128]),
                                grel_t[:, b * 2 * T:(b + 1) * 2 * T]
                                    .unsqueeze(2).broadcast_to([128, 2 * T, 128]),
                                mybir.AluOpType.is_equal)
                        else:
                            nc.vector.memset(Sap[:, 0:1, :], 0)
                    return Sap

                def aggregate(table_d, FW, out_cb, bias_wave, self_lhsT, first=False):
                    pair_ap = table_d.reshape([NPAIR, 256])[:, :]
                    for (g0, nbk) in groups:
                        g = gat.tile([128, GROUP * T, 256], fp8, tag="g")
                        ni = nbk * T * 128
                        if "gather" not in skip:
                            nc.gpsimd.dma_gather(
                                g[:, :nbk * T, :], pair_ap,
                                idx_t[:, g0 * T * 8:(g0 + nbk) * T * 8],
                                num_idxs=ni, num_idxs_reg=ni, elem_size=256,
                                single_packet=spk, queue_num=nxq())
                        else:
                            tbl3 = table_d.reshape([128, NPAD // 128 // 2, 256])
                            nc.sync.dma_start(g[:, :nbk * T, :], tbl3[:, 0:nbk * T, :])
                        selfb = self_lhsT(g0, nbk)
                        for k in range(nbk):
                            b = g0 + k
                            S = get_S(b, first)
                            psum = aps.tile([FW, 128], f32, tag="agg")
                            started = bias_wave(b, psum)
                            nc.tensor.matmul(psum[:], selfb(k), ident_t[:],
                                             start=not started, stop=False)
                            for t in range(T):
                                if "aggmm" in skip and t > 0:
                                    continue
                                last = (t == T - 1) or ("aggmm" in skip)
                                lhs = g[:, k * T + t, :].rearrange("p (j f) -> p j f", j=2)[:, :, 0:FW]
                                nc.tensor.matmul(psum[:], lhs,
                                                 S[:, 2 * t:2 * t + 2, :],
                                                 start=False, stop=last,
                                                 perf_mode=DR)
                            out_cb(b, psum)

                # ---------------- L1 aggregation (table = xp) ----------------
                def l1_out(b, psum):
                    nc.vector.tensor_copy(agg1T[:, b * 128:(b + 1) * 128], psum[:])

                def l1_self(g0, nbk):
                    xb = xbp.tile([128, GROUP, 128], fp8, tag="xb")
                    for k in range(nbk):
                        nc.sync.dma_start(xb[:, k, :],
                                          xloc_d[(g0 + k) * 128:(g0 + k + 1) * 128, :])
                    return lambda k: xb[:, k, :]

                aggregate(xp_d, 128, l1_out, lambda b, p: False, l1_self, first=True)

                # ---------------- L1 transform -> h1T ----------------
                v0 = 0
                while v0 < SLOTS:
                    vsz = min(512, SLOTS - v0)
                    for j in range(2):
                        pt = tps.tile([128, 512], f32, tag="tps")
                        nc.tensor.matmul(pt[:, :vsz], w1_t[:, j * 128:(j + 1) * 128],
                                         agg1T[:, v0:v0 + vsz], start=True, stop=False)
                        nc.tensor.matmul(pt[:, :vsz], b1_t[0:1, j * 128:(j + 1) * 128],
                                         rdegb_t[0:1, v0:v0 + vsz], start=False, stop=True)
                        h = h1T0 if j == 0 else h1T1
                        nc.scalar.activation(h[:, v0:v0 + vsz], pt[:, :vsz], RELU)
                    v0 += vsz

                # ---------------- L2 transform -> loc2/ag2in; AllGather ------
                for b in range(NB):
                    bs = slice(b * 128, (b + 1) * 128)
                    pt = tps.tile([128, 512], f32, tag="tps")
                    nc.tensor.matmul(pt[:, :128], h1T0[:, bs], w2_t[:, 0, :], start=True, stop=False)
                    nc.tensor.matmul(pt[:, :128], h1T1[:, bs], w2_t[:, 1, :], start=False, stop=True)
                    nc.scalar.activation(loc2[:, b, :], pt[:, :128], COPY,
                                         scale=dinv2p_t[:, b:b + 1])
                    nc.sync.dma_start(ag2in[bs, :], loc2[:, b, :])
                if "coll" in skip:
                    nc.sync.dma_start(ag2out[0:SLOTS, :], ag2in[:])
                else:
                    nc.gpsimd.collective_compute(
                        "AllGather", mybir.AluOpType.bypass,
                        replica_groups=rg, ins=[ag2in[:]], outs=[ag2out[:]])

                # ---------------- L2 aggregation -> h2T ----------------
                def l2_bias(b, psum):
                    nc.tensor.matmul(psum[:], b2_t[:], rdegb_t[0:1, b * 128:(b + 1) * 128],
                                     start=True, stop=False)
                    return True

                def l2_out(b, psum):
                    nc.scalar.activation(h2T[:, b * 128:(b + 1) * 128], psum[:], RELU)

                aggregate(ag2out, 128, l2_out, l2_bias,
                          lambda g0, nbk: (lambda k: loc2[:, g0 + k, :]))

                # ---------------- L3 transform -> loc3/ag3in; AllGather ------
                for b in range(NB):
                    bs = slice(b * 128, (b + 1) * 128)
                    pt = tps.tile([128, 512], f32, tag="tps")
                    nc.tensor.matmul(pt[:, :cfg.H3], h2T[:, bs], w3_t[:], start=True, stop=True)
                    nc.scalar.activation(loc3[:, b, 0:cfg.H3], pt[:, :cfg.H3], COPY,
                                         scale=dinv2p_t[:, b:b + 1])
                    nc.sync.dma_start(ag3in[bs, :], loc3[:, b, :])
                if "coll" in skip:
                    nc.sync.dma_start(ag3out[0:SLOTS, :], ag3in[:])
                else:
                    nc.gpsimd.collective_compute(
                        "AllGather", mybir.AluOpType.bypass,
                        replica_groups=rg, ins=[ag3in[:]], outs=[ag3out[:]])

                # ---------------- L3 aggregation -> h3T ----------------
                def l3_bias(b, psum):
                    nc.tensor.matmul(psum[:], b3_t[:], rdegb_t[0:1, b * 128:(b + 1) * 128],
                                     start=True, stop=False)
                    return True

                def l3_out(b, psum):
                    nc.scalar.activation(h3T[:, b * 128:(b + 1) * 128], psum[:], RELU)

                aggregate(ag3out, cfg.H3, l3_out, l3_bias,
                          lambda g0, nbk: (lambda k: loc3[:, g0 + k, 0:cfg.H3]))

                # ---------------- L4 transform -> loc4/ag4in; AllGather ------
                for b in range(NB):
                    bs = slice(b * 128, (b + 1) * 128)
                    pt = tps.tile([128, 512], f32, tag="tps")
                    nc.tensor.matmul(pt[:, :cfg.C], h3T[:, bs], w4_t[:], start=True, stop=True)
                    nc.scalar.activation(loc4[:, b, 0:cfg.C], pt[:, :cfg.C], COPY,
                                         scale=dinv2p_t[:, b:b + 1])
                    nc.sync.dma_start(ag4in[bs, :], loc4[:, b, :])
                if "coll" in skip:
                    nc.sync.dma_start(ag4out[0:SLOTS, :], ag4in[:])
                else:
                    nc.gpsimd.collective_compute(
                        "AllGather", mybir.AluOpType.bypass,
                        replica_groups=rg, ins=[ag4in[:]], outs=[ag4out[:]])

                # ---------------- L4 aggregation (node-major) -> out ----------
                pair_ap4 = ag4out.reshape([NPAIR, 256])[:, :]
                for (g0, nbk) in groups:
                    g = gat.tile([128, GROUP * T, 256], fp8, tag="g")
                    ni = nbk * T * 128
                    if "gather" not in skip:
                        nc.gpsimd.dma_gather(
                            g[:, :nbk * T, :], pair_ap4,
                            idx_t[:, g0 * T * 8:(g0 + nbk) * T * 8],
                            num_idxs=ni, num_idxs_reg=ni, elem_size=256,
                            single_packet=spk, queue_num=nxq())
                    else:
                        tbl3 = ag4out.reshape([128, NPAD // 128 // 2, 256])
                        nc.sync.dma_start(g[:, :nbk * T, :], tbl3[:, 0:nbk * T, :])
                    for k in range(nbk):
                        b = g0 + k
                        S = get_S(b, False)
                        psum = aps.tile([128, cfg.C], f32, tag="agg4")
                        nc.tensor.matmul(psum[:], rdegb_t[0:1, b * 128:(b + 1) * 128],
                                         b4_t[:], start=True, stop=False)
                        nc.tensor.matmul(psum[:], ident_t[:], loc4[:, b, 0:cfg.C],
                                         start=False, stop=False)
                        for t in range(T):
                            if "aggmm" in skip and t > 0:
                                continue
                            last = (t == T - 1) or ("aggmm" in skip)
                            rhs = g[:, k * T + t, :].rearrange("p (j f) -> p j f", j=2)[:, :, 0:cfg.C]
                            nc.tensor.matmul(psum[:], S[:, 2 * t:2 * t + 2, :],
                                             rhs, start=False, stop=last,
                                             perf_mode=DR)
                        o = epi.tile([128, cfg.C], f32, tag="o4")
                        nc.scalar.activation(o[:], psum[:], COPY, scale=dinvp_t[:, b:b + 1])
                        nc.sync.dma_start(out_d[b * 128:(b + 1) * 128, :], o[:])

            for _rep in range(reps):
                _kbody()

    nc.compile()
    return nc


# ----------------------------------------------------------------------------
# Driver
# ----------------------------------------------------------------------------

_CACHE = {}


def run(cfg, inputs, trace=False):
    in_maps, struct, inv_pos = preprocess(cfg, **inputs)
    key = (cfg.N, cfg.E, cfg.R, cfg.NB) + struct
    if key not in _CACHE:
        _CACHE[key] = build(cfg, *struct)
    nc = _CACHE[key]
    res = bass_utils.run_bass_kernel_spmd(
        nc, in_maps, core_ids=list(range(cfg.R)), trace=trace)
    outs = [res.results[r]["out"] for r in range(cfg.R)]
    full = np.concatenate(outs, axis=0)
    out = np.empty((cfg.N, cfg.C), np.float32)
    valid = inv_pos >= 0
    out[inv_pos[valid]] = full[valid]
    return out, res


def kernel(**inputs):
    out, _ = run(REAL, inputs)
    return out
